# revision 9
# baseline (speedup 1.0000x reference)
"""Trainium2 Bass kernel for per-series OLS trend extrapolation.

Math: out[b, c] = sum_w g[w] * x[b, w, c], where
  g[w] = 1/W + (w - t_mean) * (t_pred - t_mean) / sum((w - t_mean)^2)

i.e. a single fixed weighted reduction along the window axis. Pure data
parallel: batch (256) sharded 32-per-core across 8 cores.

The reduction runs on the tensor engine with K = 128 = 32 batches x 4
window-steps packed into the contraction dim (block-diagonal coef). The
PE streams rhs at 1 column/cycle, so plain fp8 runs at 128 x-elem/cycle
and the PE (not HBM) is the bottleneck. Fix: DoubleRow fp8e4 matmuls
(2 k-tiles per column, 256 x-elem/cycle, measured 216ns per 512-col
pair-matmul == 2x) for the 32 window steps with the smallest |g[w]| --
quantization error there contributes least to the output norm. The
remaining 32 large-|g| steps stay fp8_e3m4 (4 mantissa bits). DoubleRow
requires BOTH operands fp8e4, so the e4m3 coef would cost ~3.6% error;
instead the host folds g/s into x before quantization (s = signed
power of two nearest g), making the device-side coef s exactly
representable in e4m3. Emulated end-to-end rel err: 1.54e-2 (gate 2e-2,
deterministic inputs). PE time 12 units x ~1.33us = ~16us < ~18us DMA
roofline (6.4MB fp8/core at ~360 GB/s) -> memory bound, as targeted.

Schedule (per core), all driven by the ntff trace:
 - x arrives as two tensors: xa [32b, 32w, C] e3m4 (8 li-tiles) and
   xb [32b, 4pair, 2, 4wp, C] e4m3 (4 pair-tiles). Host packs w-steps
   so each li-tile's 4 partitions-per-batch read 4*C = 12.5KB
   contiguous DRAM runs.
 - two HWDGE rings (sync/scalar), ~3.2MB each, tiles enqueued in PE
   consumption order; coef tiles lead their rings. First tiles split
   into column pieces so the first matmuls start early; the last pair
   tile on each ring is split in 3 so the chunk-major tail chases the
   stream.
 - dep-free warm-up matmuls hoisted before the all-engine barrier ramp
   the PE p-state (0.65/1.2/2.4 GHz) while the first tiles stream.
 - per-chunk PSUM banks; the last two pair-units run chunk-major so
   each chunk closes early; drain copies (DVE/ACT alternating) and
   three fp16 out-DMAs chase the closes.
 - all IR blocks merged into the entry block; every cross-engine dep is
   an explicit semaphore, so BSP block handshakes are pure overhead.
"""

import numpy as np

B, W, C = 256, 64, 3142
NCORES = 8
BPC = B // NCORES       # 32 batches per core
N_E3LI = 8              # e3m4 li units (4 w-steps each)
N_PAIR = 4              # DoubleRow pair units (8 w-steps each)
NCHUNK = (C + 511) // 512
NDUMMY = 34             # PE warm-up matmuls (128 cols each)
MERGE_BLOCKS = True

_cache = {}


def _build_program():
    import concourse.bacc as bacc
    import concourse.mybir as mybir
    import concourse.tile as tile

    fp8e3 = mybir.dt.float8e3
    fp8e4 = mybir.dt.float8e4
    fp16 = mybir.dt.float16
    f32 = mybir.dt.float32
    DR = mybir.MatmulPerfMode.DoubleRow

    nc = bacc.Bacc("TRN2", target_bir_lowering=False, debug=False,
                   enable_asserts=False, num_devices=NCORES)
    xa_ap = nc.dram_tensor("xa", [BPC, 4 * N_E3LI, C], fp8e3,
                           kind="ExternalInput").ap()
    xb_ap = nc.dram_tensor("xb", [BPC, N_PAIR, 2, 4, C], fp8e4,
                           kind="ExternalInput").ap()
    ca_ap = nc.dram_tensor("coefa", [128, N_E3LI * BPC], fp16,
                           kind="ExternalInput").ap()
    cb_ap = nc.dram_tensor("coefb", [128, N_PAIR, 2, BPC], fp8e4,
                           kind="ExternalInput").ap()
    out_ap = nc.dram_tensor("out", [BPC, C], fp16, kind="ExternalOutput").ap()

    # warm-up scratch (contents irrelevant; results never read)
    warm_w = nc.alloc_sbuf_tensor("warm_w", [128, BPC], fp16).ap()
    warm_x = nc.alloc_sbuf_tensor("warm_x", [128, 128], fp8e3).ap()

    # xa li-tile: partition k = b*4 + wp holds packed step 4*li + wp
    xa_r = xa_ap.rearrange("b (li wp) c -> li b wp c", li=N_E3LI, wp=4)
    # xb pair-tile: partition k = b*4 + wp, free (i, c); step p*8 + i*4 + wp
    xb_r = xb_ap.rearrange("b p i wp c -> p b wp i c")

    with tile.TileContext(nc) as tc:
        with (
            tc.tile_pool(name="xp", bufs=1) as xp,
            tc.tile_pool(name="cp", bufs=1) as cp,
            tc.tile_pool(name="pp", bufs=1, space="PSUM") as pp,
        ):
            # PE p-state warm-up: no deps, runs right after the engine
            # prologue while the first x tiles are still streaming in
            pchunk = [pp.tile([BPC, 512], f32, name=f"ps{j}", tag=f"ps{j}")
                      for j in range(NCHUNK)]
            pwarm = pp.tile([BPC, 512], f32, name="pwarm", tag="pwarm")
            early_pe = []
            for _ in range(NDUMMY):
                di = nc.tensor.matmul(pwarm[:, :128], warm_w, warm_x,
                                      start=True, stop=True)
                early_pe.append(di.ins)

            coefA = cp.tile([128, N_E3LI * BPC], fp16, name="coefA",
                            tag="coefA")
            coefB = cp.tile([128, N_PAIR, 2, BPC], fp8e4, name="coefB",
                            tag="coefB")
            early_sync = [nc.sync.dma_start(coefB[:], cb_ap).ins]
            early_scalar = [nc.scalar.dma_start(coefA[:], ca_ap).ins]

            # column-piece splits (boundaries multiples of 512 so each
            # chunk matmul reads one piece). Pieces cost DMA efficiency
            # (they cut the 4C contiguous runs down to ~1.5-2KB), so
            # split only the first tile per ring (early PE start) and
            # the last pair tile per ring (tail chasing), once each.
            SPLITS_A = {0: (1536,), 4: (1536,)}
            SPLITS_B = {1: (2048,), 3: (2048,)}

            pieces_a = [None] * N_E3LI
            pieces_b = [None] * N_PAIR

            def load_a(li, eng, early=None):
                cuts = (0,) + SPLITS_A.get(li, ()) + (C,)
                ps = []
                for lo, hi in zip(cuts[:-1], cuts[1:]):
                    t = xp.tile([128, hi - lo], fp8e3,
                                name=f"xa{li}_{lo}", tag=f"xa{li}_{lo}")
                    di = eng.dma_start(t[:], xa_r[li][:, :, lo:hi])
                    if early is not None:
                        early.append(di.ins)
                    ps.append((t, lo, hi))
                pieces_a[li] = ps

            def load_b(p, eng):
                # two DMAs per piece (one per k-tile half i): a single
                # 4-dim (b, wp, i, c) src AP can't balance to <=3 dims
                cuts = (0,) + SPLITS_B.get(p, ()) + (C,)
                ps = []
                for lo, hi in zip(cuts[:-1], cuts[1:]):
                    t = xp.tile([128, 2, hi - lo], fp8e4,
                                name=f"xb{p}_{lo}", tag=f"xb{p}_{lo}")
                    for i in range(2):
                        eng.dma_start(t[:, i, :], xb_r[p][:, :, i, lo:hi])
                    ps.append((t, lo, hi))
                pieces_b[p] = ps

            # ring enqueue order == PE consumption order (per ring)
            load_a(0, nc.sync, early_sync)    # pieces hoisted early
            load_a(1, nc.sync)
            load_a(2, nc.sync)
            load_a(3, nc.sync)
            load_b(0, nc.sync)
            load_b(1, nc.sync)                # tail pieces
            load_a(4, nc.scalar, early_scalar)
            load_a(5, nc.scalar)
            load_a(6, nc.scalar)
            load_a(7, nc.scalar)
            load_b(2, nc.scalar)
            load_b(3, nc.scalar)              # tail pieces

            def pick(pieces, a):
                for t, lo, hi in pieces:
                    if lo <= a < hi:
                        return t, lo
                raise AssertionError

            def mm_a(li, j, **kw):
                n = min(512, C - j * 512)
                a = j * 512
                t, lo = pick(pieces_a[li], a)
                nc.tensor.matmul(pchunk[j][:, :n],
                                 coefA[:, li * BPC:(li + 1) * BPC],
                                 t[:, a - lo:a - lo + n], **kw)

            def mm_b(p, j, **kw):
                n = min(512, C - j * 512)
                a = j * 512
                t, lo = pick(pieces_b[p], a)
                nc.tensor.matmul(pchunk[j][:, :n], coefB[:, p],
                                 t[:, :, a - lo:a - lo + n],
                                 perf_mode=DR, **kw)

            # li-major phase: interleave rings in arrival order
            UNITS = [("a", 0), ("a", 4), ("a", 1), ("a", 5), ("a", 2),
                     ("a", 6), ("a", 3), ("a", 7), ("b", 0), ("b", 2)]
            for ui, (kind, idx) in enumerate(UNITS):
                for j in range(NCHUNK):
                    kw = dict(start=(ui == 0), stop=False)
                    if kind == "a":
                        mm_a(idx, j, **kw)
                    else:
                        mm_b(idx, j, **kw)
            # last two pair-units chunk-major: each chunk closes early so
            # its drain copy can chase the PE
            for j in range(NCHUNK):
                mm_b(1, j, start=False, stop=False)
                mm_b(3, j, start=False, stop=True)

            # drain: per-chunk PSUM -> SBUF(fp16) copies, all on DVE. No
            # InstActivation anywhere means no ACT table load is emitted,
            # which would otherwise sit in front of the scalar ring's
            # first DMA trigger (~1.3us).
            out_sb = cp.tile([BPC, C], fp16, name="out_sb")
            for j in range(NCHUNK):
                a, b = j * 512, min((j + 1) * 512, C)
                nc.vector.tensor_copy(out_sb[:, a:b], pchunk[j][:, :b - a])
            nc.sync.dma_start(out_ap[:, :1024], out_sb[:, :1024])
            nc.sync.dma_start(out_ap[:, 1024:2048], out_sb[:, 1024:2048])
            nc.sync.dma_start(out_ap[:, 2048:3072], out_sb[:, 2048:3072])
            nc.sync.dma_start(out_ap[:, 3072:], out_sb[:, 3072:])

    # Move the coef + first x DMA triggers (and the PE warm-up chain) ahead
    # of the all-engine barrier so they run right after the engine prologue.
    # Safe: they carry no waits, write untouched SBUF/PSUM, and their
    # completion semaphores are what the consumers already wait on.
    entry = nc.main_func.blocks[0]
    for marker, early in (
        (nc.sync.preamble_end, early_sync),
        (nc.scalar.preamble_end, early_scalar),
        (nc.tensor.preamble_end, early_pe),
    ):
        pos = entry.instructions.index(marker) + 1
        for k, ins in enumerate(early):
            assert ">=" not in str(ins), f"early ins has a wait: {ins}"
            for blk in nc.main_func.blocks:
                try:
                    blk.instructions.remove(ins)
                    break
                except ValueError:
                    continue
            entry.instructions.insert(pos + k, ins)

    if MERGE_BLOCKS:
        # Collapse the tile-context blocks into the entry block: BSP inserts
        # an all-engine drain/handshake at every block boundary; with
        # explicit semaphores carrying every cross-engine dep, the
        # boundaries are pure overhead.
        blocks = nc.main_func.blocks
        merged = []
        for bi, blk in enumerate(blocks):
            ins_list = blk.instructions
            if bi < len(blocks) - 1:
                ins_list = [i for i in ins_list
                            if not isinstance(i, mybir.InstUnconditionalBranch)]
            merged.extend(ins_list)
        entry.instructions[:] = merged
        del nc.main_func.blocks[1:]

    nc.compile()
    return nc


def _get_program():
    if "nc" not in _cache:
        _cache["nc"] = _build_program()
    return _cache["nc"]


def _plan(window: int, horizon: int):
    """Split w-steps: 32 smallest |g| -> DoubleRow e4m3 (with g folded
    into x up to a signed power-of-two coef s), 32 largest -> e3m4."""
    t = np.arange(W, dtype=np.float64)
    t_mean = (window - 1) / 2.0
    tcen = t - t_mean
    denom = (tcen * tcen).sum()
    t_pred = window + horizon - 1
    g = 1.0 / window + tcen * (t_pred - t_mean) / denom
    order = np.argsort(np.abs(g))
    ndr = 8 * N_PAIR
    dr_steps = np.sort(order[:ndr])
    e3_steps = np.sort(order[ndr:])
    gb = g[dr_steps]
    with np.errstate(divide="ignore"):
        e = np.clip(np.floor(np.log2(np.abs(gb))), -9.0, 4.0)
    s = np.where(gb == 0, 1.0, np.sign(gb) * 2.0 ** e)
    m = gb / s
    return g, e3_steps, dr_steps, s, m


def _coefs(g, e3_steps, s):
    import ml_dtypes

    ga = g[e3_steps].astype(np.float16)
    coefA = np.zeros((128, N_E3LI * BPC), np.float16)
    b = np.arange(BPC)
    for li in range(N_E3LI):
        for wp in range(4):
            coefA[b * 4 + wp, li * BPC + b] = ga[li * 4 + wp]
    coefB = np.zeros((128, N_PAIR, 2, BPC), ml_dtypes.float8_e4m3)
    s8 = s.astype(ml_dtypes.float8_e4m3)
    for p in range(N_PAIR):
        for i in range(2):
            for wp in range(4):
                coefB[b * 4 + wp, p, i, b] = s8[p * 8 + i * 4 + wp]
    return coefA, coefB


def kernel(x: np.ndarray, window, horizon) -> np.ndarray:
    import ml_dtypes
    from concourse.bass_utils import run_bass_kernel_spmd

    window = int(window)
    horizon = int(horizon)
    assert x.shape == (B, W, C), x.shape

    nc = _get_program()
    g, e3_steps, dr_steps, s, m = _plan(window, horizon)
    coefA, coefB = _coefs(g, e3_steps, s)

    x = np.ascontiguousarray(x)
    xa8 = x[:, e3_steps, :].astype(ml_dtypes.float8_e3m4)
    xb8 = (x[:, dr_steps, :] * m[None, :, None].astype(np.float32)).astype(
        ml_dtypes.float8_e4m3).reshape(B, N_PAIR, 2, 4, C)

    in_maps = [
        {
            "xa": xa8[c * BPC:(c + 1) * BPC],
            "xb": xb8[c * BPC:(c + 1) * BPC],
            "coefa": coefA,
            "coefb": coefB,
        }
        for c in range(NCORES)
    ]
    res = run_bass_kernel_spmd(nc, in_maps, list(range(NCORES)))
    out = np.concatenate([res.results[c]["out"] for c in range(NCORES)],
                         axis=0)
    return out.astype(np.float32)
